# revision 12
# baseline (speedup 1.0000x reference)
"""Causal self-attention Trainium2 kernel (8 NeuronCores).

Sharding: data-parallel over batch (2) x tensor-parallel over head groups
(12 heads -> 4 groups of 3). Core c handles batch c//4, head group c%4.
Each core computes its partial projection output; the host sums the 4
partials per batch (the TP reduce folded into the output gather).

Per-core dataflow (T=2048, C=768, local heads h0..h2, HD=64):
  host pre-transposes x -> xT [C,T] bf16 (device never transposes x)
  qkT [384,T] = Wqk_local.T @ xT   (bf16, cc-outer 8-bank accumulation
                                    so matmuls pace with the xT DMA)
  v [T,192]   = x @ Wv_local       (bf16, xT tiles as stationary lhsT)
  attention per head, two passes (tq 0:1024, 1024:2048), per k-tile j:
    S^T chunk [128, <=1024] = k_j^T q  (PSUM, exact causal widths)
    P = exp(S^T/8) -> bf16 SBUF (Scalar engine), diag 128-block masked
    on Pool with an upper-tri multiply
    yq[q] [65,512] += [v_j | ones]^T @ P chunk  (row 64 = softmax denom)
    normalize: DVE reciprocal of denom row, PE broadcast to 64 rows,
    DVE multiply -> yT bf16
  out_partial [T,C] = yT.T-slices @ Wproj_local (bf16), bf16 DMA to HBM;
  host upcasts and sums the 4 group partials per batch.

Emission is software-pipelined (PV trails QK/exp by one j) so the PE
never sits behind the Scalar exp, and warmup matmuls during the initial
DMA keep the PE p-state ramped.
"""

import functools

import ml_dtypes
import numpy as np

import concourse.bass as bass
import concourse.mybir as mybir
import concourse.tile as tile
from concourse import bacc
from concourse.bass_utils import run_bass_kernel_spmd
from concourse.masks import make_upper_triangular

P = 128
B, T, C = 2, 2048, 768
NH, HD = 12, 64
HPG = 3            # heads per core
LCH = HPG * HD     # 192 local channels
QK_CH = 2 * LCH    # 384 (q then k interleaved by pairs)
NT = T // P        # 16 t-tiles
NCC = C // P       # 6 contraction tiles
F32 = mybir.dt.float32
F32R = mybir.dt.float32r
BF16 = mybir.dt.bfloat16
VG = HD + 1        # 65: v columns + ones column per t-tile

LAST_RESULT = None


def _emit(nc, tc, xt_d, wqk_d, wv_d, wp_d, out_d):
    from contextlib import ExitStack

    ctx = ExitStack()
    with ctx:
        const = ctx.enter_context(tc.tile_pool(name="const", bufs=1))
        tri = const.tile([P, P], BF16)
        make_upper_triangular(nc, tri[:], val=1.0, diag=True)
        ones1f = const.tile([1, HD], F32)
        nc.any.memset(ones1f[:], 1.0)
        ones1 = const.tile([1, HD], F32R)
        nc.vector.tensor_copy(out=ones1[:], in_=ones1f[:])
        junk = const.tile([1, 512], BF16)
        nc.any.memset(junk[:], 1.0)
        ones1b = const.tile([1, HD], BF16)
        nc.any.memset(ones1b[:], 1.0)

        # ---------------- weights + x DMA ----------------
        w_pool = ctx.enter_context(tc.tile_pool(name="w", bufs=1))
        wqk_sb = []
        wv_sb = []
        for cc in range(NCC):
            t = w_pool.tile([P, QK_CH], BF16, tag=f"wqk{cc}")
            nc.sync.dma_start(t[:], wqk_d[cc * P : (cc + 1) * P, :])
            wqk_sb.append(t)
        for cc in range(NCC):
            t = w_pool.tile([P, LCH], BF16, tag=f"wv{cc}")
            nc.sync.dma_start(t[:], wv_d[cc * P : (cc + 1) * P, :])
            wv_sb.append(t)
        wp_a = w_pool.tile([P, C], BF16, tag="wpa")
        nc.sync.dma_start(wp_a[:], wp_d[0:P, :])
        wp_b = w_pool.tile([HD, C], BF16, tag="wpb")
        nc.sync.dma_start(wp_b[:], wp_d[P : P + HD, :])

        x_pool = ctx.enter_context(tc.tile_pool(name="x", bufs=1))
        xt_sb = []
        for cc in range(NCC):
            t = x_pool.tile([P, T], BF16, tag=f"xt{cc}")
            xt_sb.append(t)
        # DMA in halves so gen matmuls can start on the first half.
        for cc in range(NCC):
            for h in range(2):
                nc.sync.dma_start(
                    xt_sb[cc][:, h * 1024 : (h + 1) * 1024],
                    xt_d[cc * P : (cc + 1) * P, h * 1024 : (h + 1) * 1024],
                )

        # ---------------- warmup: ramp the PE p-state during DMA -------
        with tc.tile_pool(name="ps_wu", bufs=1, space="PSUM") as ps_wu:
            wt = ps_wu.tile([HD, 512], F32, tag="wu")
            for _ in range(10):
                nc.tensor.matmul(
                    wt[:], ones1b[:], junk[:], start=True, stop=True
                )

        # ---------------- qk-gen:  qkT = Wqk.T @ xT --------------------
        # M-tiles: 0 = [q0|q1], 1 = [k0|k1], 2 = [q2|k2]
        qk_pool = ctx.enter_context(tc.tile_pool(name="qk", bufs=1))
        tq01 = qk_pool.tile([P, T], BF16, tag="tq01")
        tk01 = qk_pool.tile([P, T], BF16, tag="tk01")
        tq2 = qk_pool.tile([HD, T], BF16, tag="tq2")
        tk2 = qk_pool.tile([HD, T], BF16, tag="tk2")

        # cc-outer over m in {0,1} x 4 chunks = 8 open PSUM banks; matmuls
        # chase the xT DMA halves.  m=2 runs after, full speed.
        with tc.tile_pool(name="ps_g", bufs=1, space="PSUM") as ps_g:
            ps = {}
            for m in range(2):
                for c in range(4):
                    ps[(m, c)] = ps_g.tile(
                        [P, 512], F32, tag=f"g{m}{c}", name=f"g{m}{c}"
                    )
            for cc in range(NCC):
                for c in range(4):
                    for m in range(2):
                        nc.tensor.matmul(
                            ps[(m, c)][:],
                            wqk_sb[cc][:, m * P : (m + 1) * P],
                            xt_sb[cc][:, c * 512 : (c + 1) * 512],
                            start=(cc == 0),
                            stop=(cc == NCC - 1),
                        )
            for c in range(4):
                nc.vector.tensor_copy(
                    out=tq01[:, c * 512 : (c + 1) * 512], in_=ps[(0, c)][:]
                )
                nc.vector.tensor_copy(
                    out=tk01[:, c * 512 : (c + 1) * 512], in_=ps[(1, c)][:]
                )
        with tc.tile_pool(name="ps_g2", bufs=3, space="PSUM") as ps_g2:
            for c in range(4):
                pst = ps_g2.tile([P, 512], F32, tag="g2")
                for cc in range(NCC):
                    nc.tensor.matmul(
                        pst[:],
                        wqk_sb[cc][:, 2 * P : 3 * P],
                        xt_sb[cc][:, c * 512 : (c + 1) * 512],
                        start=(cc == 0),
                        stop=(cc == NCC - 1),
                    )
                nc.vector.tensor_copy(
                    out=tq2[:, c * 512 : (c + 1) * 512], in_=pst[0:HD, :]
                )
                nc.vector.tensor_copy(
                    out=tk2[:, c * 512 : (c + 1) * 512], in_=pst[HD:P, :]
                )

        # ---------------- v-gen: v = x @ Wv (xT as stationary) ---------
        # v_sb [128, 16*195]: per t-tile 3 head groups of [64 v | 1 ones]
        v_pool = ctx.enter_context(tc.tile_pool(name="v", bufs=1))
        v_sb = v_pool.tile([P, NT * HPG * VG], BF16, tag="v")
        ones_cols = v_sb[:].rearrange("p (t g d) -> p t g d", g=HPG, d=VG)[
            :, :, :, HD:
        ]
        nc.gpsimd.memset(ones_cols, 1.0)
        with tc.tile_pool(name="ps_v", bufs=3, space="PSUM") as ps_v:
            for tt in range(NT):
                psv = ps_v.tile([P, LCH], F32, tag="pv")
                for cc in range(NCC):
                    nc.tensor.matmul(
                        psv[:],
                        xt_sb[cc][:, tt * P : (tt + 1) * P],
                        wv_sb[cc][:],
                        start=(cc == 0),
                        stop=(cc == NCC - 1),
                    )
                dst = v_sb[:, tt * HPG * VG : (tt + 1) * HPG * VG].rearrange(
                    "p (g d) -> p g d", d=VG
                )[:, :, 0:HD]
                src = psv[:].rearrange("p (g d) -> p g d", d=HD)
                nc.vector.tensor_copy(out=dst, in_=src)

        def vslice(h, jt):
            off = jt * HPG * VG + h * VG
            return v_sb[:, off : off + VG]

        # ---------------- attention ----------------
        # per head: (k tile, k offset), (q tile, q offset); bases match.
        heads = [
            (tk01, 0, tq01, 0),
            (tk01, HD, tq01, HD),
            (tk2, 0, tq2, 0),
        ]
        y_pool = ctx.enter_context(tc.tile_pool(name="y", bufs=1))
        yT_a = y_pool.tile([P, T], BF16, tag="ya")   # h0 rows 0:64, h1 64:128
        yT_b = y_pool.tile([HD, T], BF16, tag="yb")  # h2

        def ydst(h):
            return yT_a[0:HD, :] if h == 0 else (
                yT_a[HD:P, :] if h == 1 else yT_b[0:HD, :]
            )

        eb_pool = ctx.enter_context(tc.tile_pool(name="eb", bufs=3))
        rc_pool = ctx.enter_context(tc.tile_pool(name="rc", bufs=4))

        att_ctx = ExitStack()
        ps_att = att_ctx.enter_context(
            tc.tile_pool(name="ps_att", bufs=1, space="PSUM")
        )

        for h in range(HPG):
            ktile, koff, qtile, qoff = heads[h]
            kh = ktile[koff : koff + HD, :]
            qh = qtile[qoff : qoff + HD, :]
            for pas in range(2):
                lo_p = 1024 * pas          # pass tq window [lo_p, hi_p)
                hi_p = 1024 + 1024 * pas
                qlo = 2 * pas              # q-chunks {qlo, qlo+1}
                jmax = 8 * pas + 7
                yq = ps_att.tile([VG, 1024], F32, tag="yq", bufs=2, name="yq")
                # software pipeline: iteration j emits QK/exp/mask for j
                # and PV for j-1; j == jmax+1 flushes the last PV + norms.
                pend = []  # (j, eb tile, tq0 of eb)
                for j in range(jmax + 2):
                    if j <= jmax:
                        tq0 = max(lo_p, P * j)
                        w = hi_p - tq0
                        st = ps_att.tile([P, 1024], F32, tag="st", bufs=2)
                        for s0 in range(0, w, 512):
                            sw = min(512, w - s0)
                            nc.tensor.matmul(
                                st[:, s0 : s0 + sw],
                                kh[:, j * P : (j + 1) * P],
                                qh[:, tq0 + s0 : tq0 + s0 + sw],
                                start=True,
                                stop=True,
                            )
                        eb = eb_pool.tile([P, 1024], BF16, tag="eb")
                        nc.scalar.activation(
                            eb[:, 0:w],
                            st[:, 0:w],
                            mybir.ActivationFunctionType.Exp,
                            scale=0.125,
                        )
                        if P * j >= lo_p:  # diagonal block -> causal mask
                            nc.gpsimd.tensor_mul(
                                out=eb[:, 0:P], in0=eb[:, 0:P], in1=tri[:]
                            )
                        pend.append((j, eb, tq0))
                    if pend and pend[0][0] < j:
                        jj, eb, tq0 = pend.pop(0)
                        for q in (qlo, qlo + 1):
                            if q < jj // 4:
                                continue
                            lo = max(0, P * jj - 512 * q)
                            col0 = 512 * q + lo - tq0
                            qq = 512 * (q - qlo)
                            nc.tensor.matmul(
                                yq[:, qq + lo : qq + 512],
                                vslice(h, jj),
                                eb[:, col0 : col0 + 512 - lo],
                                start=(jj == 0),
                                stop=(jj == 4 * q + 3),
                            )
                # normalize the pass: denom row -> f32r sbuf -> PE bcast
                # -> reciprocal -> scale
                den = rc_pool.tile([1, 1024], F32R, tag="den")
                nc.vector.tensor_copy(out=den[:], in_=yq[HD : HD + 1, :])
                bc = ps_att.tile([HD, 1024], F32, tag="st", bufs=2, name="bc")
                for s0 in (0, 512):
                    nc.tensor.matmul(
                        bc[:, s0 : s0 + 512],
                        ones1[:],
                        den[:, s0 : s0 + 512],
                        start=True,
                        stop=True,
                    )
                bcs = rc_pool.tile([HD, 1024], F32, tag="bcs")
                with nc.allow_low_precision(reason="softmax denom"):
                    nc.vector.reciprocal_approx_fast(bcs[:], bc[:])
                nc.vector.tensor_mul(
                    out=ydst(h)[:, lo_p:hi_p],
                    in0=yq[0:HD, :],
                    in1=bcs[:],
                )

        att_ctx.close()

        # ---------------- proj: out = yT.T @ Wp (K = 128 + 64) ---------
        out_pool = ctx.enter_context(tc.tile_pool(name="outp", bufs=3))
        with tc.tile_pool(name="ps_prj", bufs=2, space="PSUM") as ps_prj:
            for tt in range(NT):
                pj = ps_prj.tile([P, C], F32, tag="pj")
                for n0, nw in ((0, 512), (512, 256)):
                    nc.tensor.matmul(
                        pj[:, n0 : n0 + nw],
                        yT_a[:, tt * P : (tt + 1) * P],
                        wp_a[:, n0 : n0 + nw],
                        start=True,
                        stop=False,
                    )
                    nc.tensor.matmul(
                        pj[:, n0 : n0 + nw],
                        yT_b[:, tt * P : (tt + 1) * P],
                        wp_b[:, n0 : n0 + nw],
                        start=False,
                        stop=True,
                    )
                ot = out_pool.tile([P, C], BF16, tag="o")
                if tt % 2 == 0:
                    nc.vector.tensor_copy(out=ot[:], in_=pj[:])
                else:
                    nc.scalar.copy(ot[:], pj[:])
                nc.sync.dma_start(out_d[tt * P : (tt + 1) * P, :], ot[:])


@functools.cache
def _build():
    nc = bacc.Bacc(
        "TRN2",
        target_bir_lowering=False,
        debug=False,
        enable_asserts=False,
        num_devices=8,
    )
    xt_d = nc.dram_tensor("xt", [C, T], BF16, kind="ExternalInput").ap()
    wqk_d = nc.dram_tensor("wqk", [C, QK_CH], BF16, kind="ExternalInput").ap()
    wv_d = nc.dram_tensor("wv", [C, LCH], BF16, kind="ExternalInput").ap()
    wp_d = nc.dram_tensor("wp", [LCH, C], BF16, kind="ExternalInput").ap()
    out_d = nc.dram_tensor("out", [T, C], BF16, kind="ExternalOutput").ap()
    with tile.TileContext(nc) as tc:
        _emit(nc, tc, xt_d, wqk_d, wv_d, wp_d, out_d)
    nc.compile()
    return nc


def kernel(x, mask, Wqkv, Wproj):
    global LAST_RESULT
    BB = ml_dtypes.bfloat16
    x = np.asarray(x, dtype=np.float32)
    Wqkv = np.asarray(Wqkv, dtype=np.float32)
    Wproj = np.asarray(Wproj, dtype=np.float32)

    def qcol(h):
        return Wqkv[:, HD * h : HD * (h + 1)]

    def kcol(h):
        return Wqkv[:, C + HD * h : C + HD * (h + 1)]

    def vcol(h):
        return Wqkv[:, 2 * C + HD * h : 2 * C + HD * (h + 1)]

    in_maps = []
    for c in range(8):
        b, g = divmod(c, 4)
        hs = [HPG * g + i for i in range(HPG)]
        # M-tiles: [q0|q1], [k0|k1], [q2|k2]
        wqk = np.concatenate(
            [qcol(hs[0]), qcol(hs[1]), kcol(hs[0]), kcol(hs[1]),
             qcol(hs[2]), kcol(hs[2])],
            axis=1,
        )
        wv = np.concatenate([vcol(hs[0]), vcol(hs[1]), vcol(hs[2])], axis=1)
        wp = Wproj[LCH * g : LCH * (g + 1), :]
        in_maps.append(
            {
                "xt": np.ascontiguousarray(x[b].T).astype(BB),
                "wqk": np.ascontiguousarray(wqk).astype(BB),
                "wv": np.ascontiguousarray(wv).astype(BB),
                "wp": np.ascontiguousarray(wp).astype(BB),
            }
        )

    nc = _build()
    res = run_bass_kernel_spmd(nc, in_maps, core_ids=list(range(8)))
    LAST_RESULT = res
    out = np.empty((B, T, C), dtype=np.float32)
    for b in range(B):
        acc = res.results[4 * b]["out"].astype(np.float32)
        for g in range(1, 4):
            acc = acc + res.results[4 * b + g]["out"].astype(np.float32)
        out[b] = acc
    return out


if __name__ == "__main__":
    rng = np.random.default_rng(0)
    x = rng.standard_normal((B, T, C), dtype=np.float32)
    wqkv = rng.standard_normal((C, 3 * C), dtype=np.float32) / np.sqrt(C)
    wproj = rng.standard_normal((C, C), dtype=np.float32) / np.sqrt(C)
    o = kernel(x, None, wqkv, wproj)
    print(o.shape, o.dtype)


# revision 16
# speedup vs baseline: 1.0860x; 1.0860x over previous
"""Causal self-attention Trainium2 kernel (8 NeuronCores).

Sharding: data-parallel over batch (2) x tensor-parallel over head groups
(12 heads -> 4 groups of 3). Core c handles batch c//4, head group c%4.
Each core computes its partial projection output; the host sums the 4
partials per batch (the TP reduce folded into the output gather).

Per-core dataflow (T=2048, C=768, local heads h0..h2, HD=64):
  host pre-transposes x -> xT [C,T] bf16 (device never transposes x)
  qkT [384,T] = Wqk_local.T @ xT   (bf16, cc-outer 8-bank accumulation
                                    so matmuls pace with the xT DMA)
  v [T,192]   = x @ Wv_local       (bf16, xT tiles as stationary lhsT)
  attention per head, two passes (tq 0:1024, 1024:2048), per k-tile j:
    S^T chunk [128, <=1024] = k_j^T q  (PSUM, exact causal widths)
    P = exp(S^T/8) -> bf16 SBUF (Scalar engine), diag 128-block masked
    on Pool with an upper-tri multiply
    yq[q] [65,512] += [v_j | ones]^T @ P chunk  (row 64 = softmax denom)
    normalize: DVE reciprocal of denom row, PE broadcast to 64 rows,
    DVE multiply -> yT bf16
  out_partial [T,C] = yT.T-slices @ Wproj_local (bf16), bf16 DMA to HBM;
  host upcasts and sums the 4 group partials per batch.

Emission is software-pipelined (PV trails QK/exp by one j) so the PE
never sits behind the Scalar exp, and warmup matmuls during the initial
DMA keep the PE p-state ramped.
"""

import functools

import ml_dtypes
import numpy as np

import concourse.bass as bass
import concourse.mybir as mybir
import concourse.tile as tile
from concourse import bacc
from concourse.bass_utils import run_bass_kernel_spmd
from concourse.masks import make_upper_triangular

P = 128
B, T, C = 2, 2048, 768
NH, HD = 12, 64
HPG = 3            # heads per core
LCH = HPG * HD     # 192 local channels
QK_CH = 2 * LCH    # 384 (q then k interleaved by pairs)
NT = T // P        # 16 t-tiles
NCC = C // P       # 6 contraction tiles
F32 = mybir.dt.float32
F32R = mybir.dt.float32r
BF16 = mybir.dt.bfloat16
VG = HD + 1        # 65: v columns + ones column per t-tile

LAST_RESULT = None


def _emit(nc, tc, xt_d, wqk_d, wv_d, wp_d, out_d):
    from contextlib import ExitStack

    ctx = ExitStack()
    with ctx:
        const = ctx.enter_context(tc.tile_pool(name="const", bufs=1))
        tri = const.tile([P, P], BF16)
        make_upper_triangular(nc, tri[:], val=1.0, diag=True)
        ones1f = const.tile([1, HD], F32)
        nc.any.memset(ones1f[:], 1.0)
        ones1 = const.tile([1, HD], F32R)
        nc.vector.tensor_copy(out=ones1[:], in_=ones1f[:])
        junk = const.tile([1, 512], BF16)
        nc.any.memset(junk[:], 1.0)
        ones1b = const.tile([1, HD], BF16)
        nc.any.memset(ones1b[:], 1.0)

        # ---------------- weights + x DMA ----------------
        w_pool = ctx.enter_context(tc.tile_pool(name="w", bufs=1))
        wqk_sb = []
        wv_sb = []
        for cc in range(NCC):
            t = w_pool.tile([P, QK_CH], BF16, tag=f"wqk{cc}")
            nc.sync.dma_start(t[:], wqk_d[cc * P : (cc + 1) * P, :])
            wqk_sb.append(t)
        for cc in range(NCC):
            t = w_pool.tile([P, LCH], BF16, tag=f"wv{cc}")
            nc.sync.dma_start(t[:], wv_d[cc * P : (cc + 1) * P, :])
            wv_sb.append(t)
        wp_a = w_pool.tile([P, C], BF16, tag="wpa")
        nc.sync.dma_start(wp_a[:], wp_d[0:P, :])
        wp_b = w_pool.tile([HD, C], BF16, tag="wpb")
        nc.sync.dma_start(wp_b[:], wp_d[P : P + HD, :])

        x_pool = ctx.enter_context(tc.tile_pool(name="x", bufs=1))
        xt_sb = []
        for cc in range(NCC):
            t = x_pool.tile([P, T], BF16, tag=f"xt{cc}")
            xt_sb.append(t)
        # DMA in halves so gen matmuls can start on the first half.
        for cc in range(NCC):
            for h in range(2):
                nc.sync.dma_start(
                    xt_sb[cc][:, h * 1024 : (h + 1) * 1024],
                    xt_d[cc * P : (cc + 1) * P, h * 1024 : (h + 1) * 1024],
                )

        # ---------------- warmup: ramp the PE p-state during DMA -------
        with tc.tile_pool(name="ps_wu", bufs=1, space="PSUM") as ps_wu:
            wt = ps_wu.tile([HD, 512], F32, tag="wu")
            for _ in range(10):
                nc.tensor.matmul(
                    wt[:], ones1b[:], junk[:], start=True, stop=True
                )

        # ---------------- qk-gen:  qkT = Wqk.T @ xT --------------------
        # M-tiles: 0 = [q0|q1], 1 = [k0|k1], 2 = [q2|k2]
        qk_pool = ctx.enter_context(tc.tile_pool(name="qk", bufs=1))
        tq01 = qk_pool.tile([P, T], BF16, tag="tq01")
        tk01 = qk_pool.tile([P, T], BF16, tag="tk01")
        tq2 = qk_pool.tile([HD, T], BF16, tag="tq2")
        tk2 = qk_pool.tile([HD, T], BF16, tag="tk2")

        # cc-outer over m in {0,1} x 4 chunks = 8 open PSUM banks; matmuls
        # chase the xT DMA halves.  m=2 runs after, full speed.
        with tc.tile_pool(name="ps_g", bufs=1, space="PSUM") as ps_g:
            ps = {}
            for m in range(2):
                for c in range(4):
                    ps[(m, c)] = ps_g.tile(
                        [P, 512], F32, tag=f"g{m}{c}", name=f"g{m}{c}"
                    )
            for cc in range(NCC):
                for c in range(4):
                    for m in range(2):
                        nc.tensor.matmul(
                            ps[(m, c)][:],
                            wqk_sb[cc][:, m * P : (m + 1) * P],
                            xt_sb[cc][:, c * 512 : (c + 1) * 512],
                            start=(cc == 0),
                            stop=(cc == NCC - 1),
                        )
            for c in range(4):
                nc.vector.tensor_copy(
                    out=tq01[:, c * 512 : (c + 1) * 512], in_=ps[(0, c)][:]
                )
                nc.vector.tensor_copy(
                    out=tk01[:, c * 512 : (c + 1) * 512], in_=ps[(1, c)][:]
                )
        # ---------------- v-gen: v = x @ Wv (xT as stationary) ---------
        # v_sb [128, 16*195]: per t-tile 3 head groups of [64 v | 1 ones]
        v_pool = ctx.enter_context(tc.tile_pool(name="v", bufs=1))
        v_sb = v_pool.tile([P, NT * HPG * VG], BF16, tag="v")
        ones_cols = v_sb[:].rearrange("p (t g d) -> p t g d", g=HPG, d=VG)[
            :, :, :, HD:
        ]
        nc.gpsimd.memset(ones_cols, 1.0)
        att_ctx = ExitStack()
        ps_att = att_ctx.enter_context(
            tc.tile_pool(name="ps_att", bufs=1, space="PSUM")
        )
        fill_ctx = ExitStack()
        ps_fill = fill_ctx.enter_context(
            tc.tile_pool(name="ps_fill", bufs=1, space="PSUM")
        )
        junk_tile = ps_fill.tile([HD, 512], F32, tag="junk")
        prj_ctx = ExitStack()
        ps_prj = None  # opened when ps_fill closes

        v_ctx = ExitStack()
        ps_v = v_ctx.enter_context(
            tc.tile_pool(name="ps_v", bufs=1, space="PSUM")
        )

        def emit_vgen(tt):
            psv = ps_v.tile([P, LCH], F32, tag="pv", name="psv")
            for cc in range(NCC):
                nc.tensor.matmul(
                    psv[:],
                    xt_sb[cc][:, tt * P : (tt + 1) * P],
                    wv_sb[cc][:],
                    start=(cc == 0),
                    stop=(cc == NCC - 1),
                )
            dst = v_sb[:, tt * HPG * VG : (tt + 1) * HPG * VG].rearrange(
                "p (g d) -> p g d", d=VG
            )[:, :, 0:HD]
            src = psv[:].rearrange("p (g d) -> p g d", d=HD)
            nc.vector.tensor_copy(out=dst, in_=src)

        for tt in range(8):
            emit_vgen(tt)

        def vslice(h, jt):
            off = jt * HPG * VG + h * VG
            return v_sb[:, off : off + VG]

        # ---------------- attention ----------------
        heads = [
            (tk01, 0, tq01, 0),
            (tk01, HD, tq01, HD),
            (tk2, 0, tq2, 0),
        ]
        y_pool = ctx.enter_context(tc.tile_pool(name="y", bufs=1))
        yT_a = y_pool.tile([P, T], BF16, tag="ya")   # h0 rows 0:64, h1 64:128
        yT_b = y_pool.tile([HD, T], BF16, tag="yb")  # h2

        def ydst(h):
            return yT_a[0:HD, :] if h == 0 else (
                yT_a[HD:P, :] if h == 1 else yT_b[0:HD, :]
            )

        eb_pool = ctx.enter_context(tc.tile_pool(name="eb", bufs=4))
        rc_pool = ctx.enter_context(tc.tile_pool(name="rc", bufs=4))
        out_pool = ctx.enter_context(tc.tile_pool(name="outp", bufs=3))


        # ---- filler engine: real PE work (v-gen tail, q2/k2 gen, proj)
        # injected between attention groups; junk matmuls as fallback so
        # the PE p-state never drops.
        m2_ctx = ExitStack()
        pools = {}
        m2ps = {}
        ot_tiles = {}

        def emit_m2(c, cc):
            if cc == 0:
                m2ps[c] = pools["m2"].tile(
                    [P, 512], F32, tag="m2", name="m2ps"
                )
            nc.tensor.matmul(
                m2ps[c][:],
                wqk_sb[cc][:, 2 * P : 3 * P],
                xt_sb[cc][:, c * 512 : (c + 1) * 512],
                start=(cc == 0),
                stop=(cc == NCC - 1),
            )
            if cc == NCC - 1:
                nc.vector.tensor_copy(
                    out=tq2[:, c * 512 : (c + 1) * 512], in_=m2ps[c][0:HD, :]
                )
                nc.vector.tensor_copy(
                    out=tk2[:, c * 512 : (c + 1) * 512], in_=m2ps[c][HD:P, :]
                )

        def emit_proj(tt, n0, tail=False):
            nw = 256 if n0 else 512
            pj = ps_prj.tile([P, nw], F32, tag="pj", bufs=2, name="pj")
            nc.tensor.matmul(
                pj[:],
                yT_a[:, tt * P : (tt + 1) * P],
                wp_a[:, n0 : n0 + nw],
                start=True,
                stop=False,
            )
            nc.tensor.matmul(
                pj[:],
                yT_b[:, tt * P : (tt + 1) * P],
                wp_b[:, n0 : n0 + nw],
                start=False,
                stop=True,
            )
            if n0 == 0:
                ot_tiles[tt] = out_pool.tile([P, C], BF16, tag="o", name="ot")
            if tail and n0:
                nc.scalar.copy(ot_tiles[tt][:, n0 : n0 + nw], pj[:])
            else:
                nc.vector.tensor_copy(
                    out=ot_tiles[tt][:, n0 : n0 + nw], in_=pj[:]
                )
            if n0:
                nc.sync.dma_start(
                    out_d[tt * P : (tt + 1) * P, :], ot_tiles[tt][:]
                )

        class Filler:
            def __init__(self):
                self.items = []
                self.deficit = 0.0
                self.junk_ok = True

            def earn(self, ns):
                self.deficit = min(self.deficit + ns, 3500.0)

            def emit_one(self, it):
                kind = it[0]
                if kind == "v":
                    emit_vgen(it[1])
                    return 900.0
                if kind == "m2":
                    emit_m2(it[1], it[2])
                    return 275.0
                if kind == "proj":
                    emit_proj(it[1], it[2])
                    return 560.0
                return 0.0

            def run(self):
                while self.deficit > 250.0:
                    if self.items:
                        self.deficit -= self.emit_one(self.items.pop(0))
                    elif self.junk_ok:
                        nc.tensor.matmul(
                            junk_tile[:], ones1b[:], junk[:],
                            start=True, stop=True,
                        )
                        self.deficit -= 240.0
                    else:
                        break

            def flush_real(self):
                for it in self.items:
                    self.emit_one(it)
                self.items = []

        filler = Filler()

        def row_groups(pas):
            lo_p, hi_p = 1024 * pas, 1024 + 1024 * pas
            groups, cur, cw = [], [], 0
            for j in range(0, 8 * pas + 8):
                tq0 = max(lo_p, P * j)
                w = hi_p - tq0
                if cur and cw + w > 1024:
                    groups.append(cur)
                    cur, cw = [], 0
                cur.append((j, cw, tq0, w))
                cw += w
            groups.append(cur)
            return groups

        for h in range(HPG):
            ktile, koff, qtile, qoff = heads[h]
            kh = ktile[koff : koff + HD, :]
            qh = qtile[qoff : qoff + HD, :]
            if h == 0:
                filler.items = [("v", tt) for tt in range(8, NT)]
            elif h == 1:
                v_ctx.close()
                pools["m2"] = m2_ctx.enter_context(
                    tc.tile_pool(name="ps_m2", bufs=1, space="PSUM")
                )
                filler.items = [
                    ("m2", c, cc) for c in range(4) for cc in range(NCC)
                ]
            for pas in range(2):
                lo_p, hi_p = 1024 * pas, 1024 + 1024 * pas
                qlo = 2 * pas
                if h == 0 and pas == 1:
                    # pass2 PV reads v tiles 8..15 - they must exist
                    filler.flush_real()
                if h == 2 and pas == 1:
                    # proj of the first half fills h2/pass2; junk pool
                    # closes to free PSUM for the proj accumulators.
                    filler.flush_real()
                    m2_ctx.close()
                    fill_ctx.close()
                    ps_prj = prj_ctx.enter_context(
                        tc.tile_pool(name="ps_prj", bufs=1, space="PSUM")
                    )
                    filler.items = [
                        ("proj", tt, n0) for tt in range(8) for n0 in (0, 512)
                    ]
                    filler.junk_ok = False
                yq = ps_att.tile([VG, 1024], F32, tag="yq", bufs=1, name="yq")
                pend = None
                for g in row_groups(pas) + [None]:
                    if g is not None:
                        gw = sum(x[3] for x in g)
                        st = ps_att.tile(
                            [P, 1024], F32, tag="st", bufs=2, name="st"
                        )
                        for j, off, tq0, w in g:
                            for s0 in range(0, w, 512):
                                sw = min(512, w - s0)
                                nc.tensor.matmul(
                                    st[:, off + s0 : off + s0 + sw],
                                    kh[:, j * P : (j + 1) * P],
                                    qh[:, tq0 + s0 : tq0 + s0 + sw],
                                    start=True,
                                    stop=True,
                                )
                                filler.deficit -= 0.42 * sw + 60
                        eb = eb_pool.tile([P, 1024], BF16, tag="eb", name="eb")
                        nc.scalar.activation(
                            eb[:, 0:gw],
                            st[:, 0:gw],
                            mybir.ActivationFunctionType.Exp,
                            scale=0.125,
                        )
                        filler.earn(0.84 * gw + 420)
                        for j, off, tq0, w in g:
                            if tq0 == P * j:
                                nc.gpsimd.tensor_mul(
                                    out=eb[:, off : off + P],
                                    in0=eb[:, off : off + P],
                                    in1=tri[:],
                                )
                    filler.run()
                    if pend is not None:
                        peb, pg = pend
                        for j, off, tq0, w in pg:
                            for q in (qlo, qlo + 1):
                                if q < j // 4:
                                    continue
                                lo = max(0, P * j - 512 * q)
                                col = off + 512 * q + lo - tq0
                                qq = 512 * (q - qlo)
                                nc.tensor.matmul(
                                    yq[:, qq + lo : qq + 512],
                                    vslice(h, j),
                                    peb[:, col : col + 512 - lo],
                                    start=(j == 0),
                                    stop=(j == 4 * q + 3),
                                )
                                filler.deficit -= 0.42 * (512 - lo) + 60
                    pend = (eb, g) if g is not None else None
                # normalize the pass
                den = rc_pool.tile([1, 1024], F32R, tag="den")
                nc.vector.tensor_copy(out=den[:], in_=yq[HD : HD + 1, :])
                bc = ps_att.tile(
                    [HD, 1024], F32, tag="st", bufs=2, name="bc"
                )
                for s0 in (0, 512):
                    nc.tensor.matmul(
                        bc[:, s0 : s0 + 512],
                        ones1[:],
                        den[:, s0 : s0 + 512],
                        start=True,
                        stop=True,
                    )
                bcs = rc_pool.tile([HD, 1024], F32, tag="bcs")
                with nc.allow_low_precision(reason="softmax denom"):
                    nc.vector.reciprocal_approx_fast(bcs[:], bc[:])
                nc.vector.tensor_mul(
                    out=ydst(h)[:, lo_p:hi_p],
                    in0=yq[0:HD, :],
                    in1=bcs[:],
                )
            filler.flush_real()

        # ---------------- proj tail: remaining t-tiles ------------------
        for tt in range(8, NT):
            for n0 in (0, 512):
                emit_proj(tt, n0, tail=True)
        prj_ctx.close()
        att_ctx.close()


@functools.cache
def _build():
    nc = bacc.Bacc(
        "TRN2",
        target_bir_lowering=False,
        debug=False,
        enable_asserts=False,
        num_devices=8,
    )
    xt_d = nc.dram_tensor("xt", [C, T], BF16, kind="ExternalInput").ap()
    wqk_d = nc.dram_tensor("wqk", [C, QK_CH], BF16, kind="ExternalInput").ap()
    wv_d = nc.dram_tensor("wv", [C, LCH], BF16, kind="ExternalInput").ap()
    wp_d = nc.dram_tensor("wp", [LCH, C], BF16, kind="ExternalInput").ap()
    out_d = nc.dram_tensor("out", [T, C], BF16, kind="ExternalOutput").ap()
    with tile.TileContext(nc) as tc:
        _emit(nc, tc, xt_d, wqk_d, wv_d, wp_d, out_d)
    nc.compile()
    return nc


def kernel(x, mask, Wqkv, Wproj):
    global LAST_RESULT
    BB = ml_dtypes.bfloat16
    x = np.asarray(x, dtype=np.float32)
    Wqkv = np.asarray(Wqkv, dtype=np.float32)
    Wproj = np.asarray(Wproj, dtype=np.float32)

    def qcol(h):
        return Wqkv[:, HD * h : HD * (h + 1)]

    def kcol(h):
        return Wqkv[:, C + HD * h : C + HD * (h + 1)]

    def vcol(h):
        return Wqkv[:, 2 * C + HD * h : 2 * C + HD * (h + 1)]

    in_maps = []
    for c in range(8):
        b, g = divmod(c, 4)
        hs = [HPG * g + i for i in range(HPG)]
        # M-tiles: [q0|q1], [k0|k1], [q2|k2]
        wqk = np.concatenate(
            [qcol(hs[0]), qcol(hs[1]), kcol(hs[0]), kcol(hs[1]),
             qcol(hs[2]), kcol(hs[2])],
            axis=1,
        )
        wv = np.concatenate([vcol(hs[0]), vcol(hs[1]), vcol(hs[2])], axis=1)
        wp = Wproj[LCH * g : LCH * (g + 1), :]
        in_maps.append(
            {
                "xt": np.ascontiguousarray(x[b].T).astype(BB),
                "wqk": np.ascontiguousarray(wqk).astype(BB),
                "wv": np.ascontiguousarray(wv).astype(BB),
                "wp": np.ascontiguousarray(wp).astype(BB),
            }
        )

    nc = _build()
    res = run_bass_kernel_spmd(nc, in_maps, core_ids=list(range(8)))
    LAST_RESULT = res
    out = np.empty((B, T, C), dtype=np.float32)
    for b in range(B):
        acc = res.results[4 * b]["out"].astype(np.float32)
        for g in range(1, 4):
            acc = acc + res.results[4 * b + g]["out"].astype(np.float32)
        out[b] = acc
    return out


if __name__ == "__main__":
    rng = np.random.default_rng(0)
    x = rng.standard_normal((B, T, C), dtype=np.float32)
    wqkv = rng.standard_normal((C, 3 * C), dtype=np.float32) / np.sqrt(C)
    wproj = rng.standard_normal((C, C), dtype=np.float32) / np.sqrt(C)
    o = kernel(x, None, wqkv, wproj)
    print(o.shape, o.dtype)
